# revision 8
# baseline (speedup 1.0000x reference)
"""CNNMambaFast Trainium2 kernel.

Layout: features-on-partitions, time-on-free everywhere (the input [M, T],
the selective scan, and the output [NC, T] are all naturally "transposed",
so the whole network runs with zero transposes).

Sharding: cores (2b, 2b+1) both compute sample b's token-side work in full
(LN, projections, conv); the 16 SSM states are split 8/8 between them.  The
n-half asymmetry is carried entirely by per-core input data (A columns,
xproj B/C rows, halved D_skip and residual).  One pair AllReduce of the
[DM, T] partial per layer recombines the halves.

The scan runs on the vector engine via tensor_tensor_scan
(state = eA * state + b along the free/time axis) with exp(A*delta) fused
into one scalar-engine activation per (state, channel-block) tile; B/C
broadcasts across partitions are row-selector matmuls on the tensor engine.
"""

import sys

sys.path.insert(0, '/opt/trn_rl_repo')

import numpy as np

import concourse.bass as bass
import concourse.mybir as mybir
from concourse import tile
from concourse.bass_utils import run_bass_kernel_spmd
from concourse.vector_clock import ScopedClock, VectorClock

F32 = mybir.dt.float32
AO = mybir.AluOpType
AF = mybir.ActivationFunctionType

B, M, T = 4, 128, 1024
NL, DM, DI = 4, 512, 1024
N, DC, DR = 16, 4, 32
NCLS, ENC, CD = 10, 512, 128
ND = 2 * M
EPS = 1e-5
W = 512           # time-chunk width
NCH = T // W      # chunks
NH = N // 2       # states per core
KD = DI // 128    # d-inner partition blocks (8)
KM = DM // 128    # d-model partition blocks (4)


# ---------------------------------------------------------------- fixups ---
class _SplitDrainTC(tile.TileContext):
    """Exit drain's sem waits go one-per-nop: this walrus build rejects more
    than one sync wait on a single instruction."""

    def _drain_and_barrier(self, tick_clock, wait_clock):
        gc = tick_clock.global_clock
        nz = [i for i in range(len(gc)) if gc[i] > 0]
        for p in nz:
            sub = [0] * len(gc)
            sub[p] = gc[p]
            nop = self.nc.sync.nop(nofuse=True, hint="split_drain_wait").ins
            wait_clock.add_sem_waits(nop, ScopedClock({None: VectorClock(sub)}))
        self.nc.sync.drain()
        self.nc.all_engine_barrier()
        assert self.sems is not None
        popped = self.nc._tile_sem_poison_stack.pop()
        assert popped is self._sem_poison
        self.nc.clear_and_free_semaphores(list(self.sems.allocated().values()))
        self.nc.all_engine_barrier()


def _split_sync_waits(nc, max_waits=1):
    """Move excess per-instruction sem waits onto same-engine NoOps."""
    n_split = 0
    for fn in nc.m.functions:
        for blk in fn.blocks:
            out = []
            for inst in blk.instructions:
                si = getattr(inst, "sync_info", None)
                if si is not None and si.on_wait and len(si.on_wait) > max_waits:
                    waits = list(si.on_wait)
                    keep = waits[-max_waits:]
                    extra = waits[:-max_waits]
                    for i in range(0, len(extra), max_waits):
                        nop = mybir.InstNoOp(
                            name=nc.get_next_instruction_name(),
                            sync_info=mybir.SyncInfo(
                                on_wait=extra[i:i + max_waits], on_update=[]),
                        )
                        nop.engine = inst.engine
                        out.append(nop)
                        n_split += 1
                    si.on_wait = keep
                out.append(inst)
            blk.instructions[:] = out
    return n_split


# ------------------------------------------------------------ bias packing ---
_BP = {}


def _bp_alloc():
    i = 0

    def take(name, n):
        nonlocal i
        _BP[name] = i
        i += n

    take('fc1_b', 4); take('fc2_b', 2); take('proj_b', 4); take('h1_b', 1)
    take('ln_g', NL * KM); take('ln_b', NL * KM)
    take('hn_g', KM); take('hn_b', KM); take('eps', 1)
    return i


_NBC = _bp_alloc()


def _bpcol(name, j=0):
    return _BP[name] + j


# ----------------------------------------------------------------- builder ---
def build_nc():
    nc = bass.Bass(num_devices=8)

    dt = nc.dram_tensor
    d_in = {
        'xb': dt('xb', [M, T], F32, kind="ExternalInput"),
        'fc1_wT': dt('fc1_wT', [ND, ENC], F32, kind="ExternalInput"),
        'fc2_wT': dt('fc2_wT', [ENC, ND], F32, kind="ExternalInput"),
        'proj_wT': dt('proj_wT', [ND, DM], F32, kind="ExternalInput"),
        'bias_pack': dt('bias_pack', [128, _NBC], F32, kind="ExternalInput"),
        'h1_wT': dt('h1_wT', [DM, CD], F32, kind="ExternalInput"),
        'h2_wT': dt('h2_wT', [CD, NCLS], F32, kind="ExternalInput"),
        'h2_b': dt('h2_b', [NCLS, 1], F32, kind="ExternalInput"),
        'selmat': dt('selmat', [48, 16 * 128], F32, kind="ExternalInput"),
    }
    for l in range(NL):
        d_in[f'inwT{l}'] = dt(f'inwT{l}', [DM, 2 * DI], F32, kind="ExternalInput")
        d_in[f'outwT{l}'] = dt(f'outwT{l}', [DI, DM], F32, kind="ExternalInput")
        d_in[f'xpjT{l}'] = dt(f'xpjT{l}', [DI, 48], F32, kind="ExternalInput")
        d_in[f'dtwT{l}'] = dt(f'dtwT{l}', [DR, DI], F32, kind="ExternalInput")
        d_in[f'cpak{l}'] = dt(f'cpak{l}', [DI, 7], F32, kind="ExternalInput")
        d_in[f'Ah{l}'] = dt(f'Ah{l}', [DI, NH], F32, kind="ExternalInput")
    d_out = dt('out', [NCLS, T], F32, kind="ExternalOutput")

    with _SplitDrainTC(nc) as tc:
        with tc.tile_pool(name="wc", bufs=1) as wc, \
             tc.tile_pool(name="ch", bufs=2) as ch, \
             tc.tile_pool(name="psA", bufs=1, space="PSUM") as psA, \
             tc.tile_pool(name="psB", bufs=2, space="PSUM") as psB, \
             tc.tile_pool(name="psC", bufs=1, space="PSUM") as psC, \
             tc.tile_pool(name="psD", bufs=4, space="PSUM") as psD, \
             tc.tile_pool(name="dram", bufs=4, space="DRAM") as drp:

            BF16 = mybir.dt.bfloat16

            def ctile(tag, bufs, shape=(128, W), dtype=F32):
                return ch.tile(list(shape), dtype, tag=tag, bufs=bufs, name=tag)

            # ---------------- constants / persistent weights ----------------
            ones_col = wc.tile([128, 1], F32, tag="ones_col", name="ones_col")
            nc.vector.memset(ones_col[:], 1.0)
            ones_row = wc.tile([1, 128], F32, tag="ones_row", name="ones_row")
            nc.vector.memset(ones_row[:], 1.0)
            bp = wc.tile([128, _NBC], F32, tag="bp", name="bp")
            nc.sync.dma_start(bp[:], d_in['bias_pack'][:])

            def bcol(name, j=0):
                return bp[:, _bpcol(name, j):_bpcol(name, j) + 1]

            selm = wc.tile([48, 16 * 128], F32, tag="selm", name="selm")
            nc.sync.dma_start(selm[:], d_in['selmat'][:])
            selB = [selm[:, 128 * n:128 * (n + 1)] for n in range(NH)]
            selC = [selm[:, 128 * (8 + n):128 * (9 + n)] for n in range(NH)]

            fc1w = [wc.tile([128, ENC], F32, tag=f"fc1w{k}", name=f"fc1w{k}") for k in range(2)]
            for k in range(2):
                nc.sync.dma_start(fc1w[k][:], d_in['fc1_wT'][128 * k:128 * (k + 1), :])
            fc2w = [wc.tile([128, ND], F32, tag=f"fc2w{k}", name=f"fc2w{k}") for k in range(4)]
            for k in range(4):
                nc.sync.dma_start(fc2w[k][:], d_in['fc2_wT'][128 * k:128 * (k + 1), :])
            projw = [wc.tile([128, DM], F32, tag=f"projw{k}", name=f"projw{k}") for k in range(2)]
            for k in range(2):
                nc.sync.dma_start(projw[k][:], d_in['proj_wT'][128 * k:128 * (k + 1), :])
            h1w = [wc.tile([128, CD], F32, tag=f"h1w{k}", name=f"h1w{k}") for k in range(4)]
            for k in range(4):
                nc.sync.dma_start(h1w[k][:], d_in['h1_wT'][128 * k:128 * (k + 1), :])
            h2w = wc.tile([CD, NCLS], F32, tag="h2w", name="h2w")
            nc.sync.dma_start(h2w[:], d_in['h2_wT'][:])
            h2b = wc.tile([NCLS, 1], F32, tag="h2b", name="h2b")
            nc.sync.dma_start(h2b[:], d_in['h2_b'][:])

            h_dram = drp.tile([DM, T], F32, tag="cc", name="cc")

            # ---------------- encoder (chunked) ----------------
            for c in range(NCH):
                sl = c * W
                f0 = ctile("u", 4)
                nc.sync.dma_start(f0[:], d_in['xb'][:, sl:sl + W])
                f1 = ctile("u", 4)
                if c == 0:
                    nc.vector.memset(f1[:, 0:1], 0.0)
                else:
                    xc = ctile("xcol", 2, (128, 1))
                    nc.sync.dma_start(xc[:], d_in['xb'][:, sl - 1:sl])
                    nc.vector.tensor_tensor(f1[:, 0:1], f0[:, 0:1], xc[:],
                                            AO.subtract)
                nc.vector.tensor_tensor(f1[:, 1:W], f0[:, 1:W], f0[:, 0:W - 1],
                                        AO.subtract)
                nc.vector.tensor_scalar_max(f1[:, 1:W], f1[:, 1:W], 0.0)
                if c > 0:
                    nc.vector.tensor_scalar_max(f1[:, 0:1], f1[:, 0:1], 0.0)
                feat = [f0, f1]
                e1 = []
                for mi in range(4):
                    p = psA.tile([128, W], F32, tag="mm", name="mm")
                    for k in range(2):
                        nc.tensor.matmul(p[:], fc1w[k][:, 128 * mi:128 * (mi + 1)],
                                         feat[k][:], start=(k == 0), stop=(k == 1))
                    t = ctile("uc", 8)
                    nc.scalar.activation(t[:], p[:], AF.Silu, bias=bcol('fc1_b', mi))
                    e1.append(t)
                e2 = []
                for mi in range(2):
                    p = psA.tile([128, W], F32, tag="mm", name="mm")
                    for k in range(4):
                        nc.tensor.matmul(p[:], fc2w[k][:, 128 * mi:128 * (mi + 1)],
                                         e1[k][:], start=(k == 0), stop=(k == 3))
                    t = ctile("uc", 8)
                    nc.vector.tensor_scalar(t[:], p[:], bcol('fc2_b', mi), None,
                                            AO.add)
                    e2.append(t)
                for mi in range(KM):
                    p = psA.tile([128, W], F32, tag="mm", name="mm")
                    for k in range(2):
                        nc.tensor.matmul(p[:], projw[k][:, 128 * mi:128 * (mi + 1)],
                                         e2[k][:], start=(k == 0), stop=(k == 1))
                    t = ctile("pre", 2)
                    nc.vector.tensor_scalar(t[:], p[:], bcol('proj_b', mi), None,
                                            AO.add)
                    nc.sync.dma_start(h_dram[128 * mi:128 * (mi + 1), sl:sl + W],
                                      t[:])

            # ---------------- layer-norm helper (chunked) ----------------
            def layernorm(hch, gname, bname, lidx, out_tag):
                sq_p = psA.tile([1, W], F32, tag="mm", name="mm")
                m1_p = psB.tile([128, W], F32, tag="bc", name="bc")
                # m1 into psB[1,W] view? separate tags sized [128,W]; use rows.
                for j in range(KM):
                    sq = ctile("lnt1", 4)
                    nc.vector.tensor_tensor(sq[:], hch[j][:], hch[j][:], AO.mult)
                    nc.tensor.matmul(sq_p[:], ones_col[:], sq[:],
                                     start=(j == 0), stop=(j == KM - 1))
                    nc.tensor.matmul(m1_p[0:1, :], ones_col[:], hch[j][:],
                                     start=(j == 0), stop=(j == KM - 1))
                meanr = ctile("stat", 6, (1, W))
                nc.vector.tensor_scalar_mul(meanr[:], m1_p[0:1, :], 1.0 / DM)
                msq = ctile("stat", 6, (1, W))
                nc.vector.tensor_tensor(msq[:], meanr[:], meanr[:], AO.mult)
                varr = ctile("stat", 6, (1, W))
                nc.vector.scalar_tensor_tensor(varr[:], sq_p[:], 1.0 / DM, msq[:],
                                               AO.mult, AO.subtract)
                lnv = ctile("stat", 6, (1, W))
                nc.scalar.activation(lnv[:], varr[:], AF.Ln,
                     bias=bp[0:1, _bpcol('eps'):_bpcol('eps') + 1])
                rstd = ctile("stat", 6, (1, W))
                nc.scalar.activation(rstd[:], lnv[:], AF.Exp, scale=-0.5)
                mub = psB.tile([128, W], F32, tag="bc", name="bc")
                nc.tensor.matmul(mub[:], ones_row[:], meanr[:], start=True,
                                 stop=True)
                rob = psB.tile([128, W], F32, tag="bc", name="bc")
                nc.tensor.matmul(rob[:], ones_row[:], rstd[:], start=True,
                                 stop=True)
                outs = []
                for j in range(KM):
                    t1 = ctile("lnt1", 4)
                    nc.vector.tensor_tensor(t1[:], hch[j][:], mub[:], AO.subtract)
                    nc.vector.tensor_tensor(t1[:], t1[:], rob[:], AO.mult)
                    o = ctile(out_tag, 4)
                    gc_ = bcol(gname, lidx * KM + j) if gname == 'ln_g' \
                        else bcol(gname, j)
                    bc_ = bcol(bname, lidx * KM + j) if bname == 'ln_b' \
                        else bcol(bname, j)
                    nc.vector.tensor_scalar(o[:], t1[:], gc_, bc_, AO.mult, AO.add)
                    outs.append(o)
                return outs

            # ---------------- layers ----------------
            hsrc = h_dram
            for l in range(NL):
                inw = [wc.tile([128, 2 * DI], F32, tag=f"inw{k}", name=f"inw{k}") for k in range(KM)]
                for k in range(KM):
                    nc.sync.dma_start(inw[k][:],
                                      d_in[f'inwT{l}'][128 * k:128 * (k + 1), :])
                outw = [wc.tile([128, DM], F32, tag=f"outw{k}", name=f"outw{k}") for k in range(KD)]
                for k in range(KD):
                    nc.sync.dma_start(outw[k][:],
                                      d_in[f'outwT{l}'][128 * k:128 * (k + 1), :])
                xpj = [wc.tile([128, 48], F32, tag=f"xpj{k}", name=f"xpj{k}") for k in range(KD)]
                for k in range(KD):
                    nc.sync.dma_start(xpj[k][:],
                                      d_in[f'xpjT{l}'][128 * k:128 * (k + 1), :])
                dtw = wc.tile([DR, DI], F32, tag="dtw", name="dtw")
                nc.sync.dma_start(dtw[:], d_in[f'dtwT{l}'][:])
                cpk = [wc.tile([128, 7], F32, tag=f"cpk{j}", name=f"cpk{j}") for j in range(KD)]
                for j in range(KD):
                    nc.sync.dma_start(cpk[j][:],
                                      d_in[f'cpak{l}'][128 * j:128 * (j + 1), :])
                Ah = [wc.tile([128, NH], F32, tag=f"Ah{j}", name=f"Ah{j}") for j in range(KD)]
                for j in range(KD):
                    nc.sync.dma_start(Ah[j][:],
                                      d_in[f'Ah{l}'][128 * j:128 * (j + 1), :])

                hstate = [ctile("hstate", 16, (128, NH)) for _ in range(KD)]
                halo = [None] * KD
                cc_in = drp.tile([DM, T], F32, tag="cc", name="cc")

                for c in range(NCH):
                    sl = c * W
                    hch = []
                    for j in range(KM):
                        t = ctile("hch", 5)
                        nc.sync.dma_start(t[:],
                                          hsrc[128 * j:128 * (j + 1), sl:sl + W])
                        hch.append(t)
                    hn = layernorm(hch, 'ln_g', 'ln_b', l, "hn")

                    # in_proj; u-half interleaved with conv, then z-half
                    zs = []
                    uc = []
                    for j in range(KD):
                        p = psA.tile([128, W], F32, tag="mm", name="mm")
                        for k in range(KM):
                            nc.tensor.matmul(p[:], inw[k][:, 128 * j:128 * (j + 1)],
                                             hn[k][:], start=(k == 0),
                                             stop=(k == KM - 1))
                        ut = ctile("u", 4)
                        nc.vector.tensor_copy(ut[:], p[:])
                        # causal depthwise conv (taps into previous chunk's halo)
                        ucp = ctile("ucp", 2)
                        nc.vector.tensor_scalar_mul(ucp[:], ut[:], cpk[j][:, 3:4])
                        for s in (1, 2, 3):
                            nc.vector.scalar_tensor_tensor(
                                ucp[:, s:W], ut[:, 0:W - s], cpk[j][:, 3 - s:4 - s],
                                ucp[:, s:W], AO.mult, AO.add)
                            if c > 0:
                                nc.vector.scalar_tensor_tensor(
                                    ucp[:, 0:s], halo[j][:, 3 - s:3],
                                    cpk[j][:, 3 - s:4 - s], ucp[:, 0:s],
                                    AO.mult, AO.add)
                        nh_t = ctile("halo", 16, (128, 3))
                        nc.vector.tensor_copy(nh_t[:], ut[:, W - 3:W])
                        halo[j] = nh_t
                        uct = ctile("uc", 8)
                        nc.scalar.activation(uct[:], ucp[:], AF.Silu,
                                             bias=cpk[j][:, 4:5])
                        uc.append(uct)
                    for j in range(KD):
                        p = psA.tile([128, W], F32, tag="mm", name="mm")
                        for k in range(KM):
                            nc.tensor.matmul(p[:],
                                             inw[k][:, DI + 128 * j:DI + 128 * (j + 1)],
                                             hn[k][:], start=(k == 0),
                                             stop=(k == KM - 1))
                        zt = ctile("zs", 8, dtype=BF16)
                        nc.scalar.activation(zt[:], p[:], AF.Silu)
                        zs.append(zt)

                    # xproj -> dbl [48, W]  (rows: dt 0:32, B 32:40, C 40:48)
                    pxp = psC.tile([48, W], F32, tag="xp", name="xp")
                    for k in range(KD):
                        nc.tensor.matmul(pxp[:], xpj[k][:], uc[k][:],
                                         start=(k == 0), stop=(k == KD - 1))
                    dbl = ctile("dbl", 2, (48, W))
                    nc.vector.tensor_copy(dbl[:], pxp[:])

                    # scan (j outer; out_proj accumulates into psD across j)
                    psd = [psD.tile([128, W], F32, tag="od", name="od") for _ in range(KM)]
                    for j in range(KD):
                        p = psA.tile([128, W], F32, tag="mm", name="mm")
                        nc.tensor.matmul(p[:], dtw[:, 128 * j:128 * (j + 1)],
                                         dbl[0:DR, :], start=True, stop=True)
                        ex = ctile("ex", 3)
                        nc.scalar.activation(ex[:], p[:], AF.Exp,
                                             bias=cpk[j][:, 5:6])
                        dl = ctile("ex", 3)
                        nc.scalar.activation(dl[:], ex[:], AF.Ln, bias=1.0)
                        du_t = ctile("du", 2)
                        nc.vector.tensor_tensor(du_t[:], dl[:], uc[j][:], AO.mult)
                        y_t = ctile("y", 2)
                        nc.vector.tensor_scalar_mul(y_t[:], uc[j][:],
                                                    cpk[j][:, 6:7])
                        for n in range(NH):
                            Bb = psB.tile([128, W], F32, tag="bc", name="bc")
                            nc.tensor.matmul(Bb[:], selB[n], dbl[:],
                                             start=True, stop=True)
                            Cb = psB.tile([128, W], F32, tag="bc", name="bc")
                            nc.tensor.matmul(Cb[:], selC[n], dbl[:],
                                             start=True, stop=True)
                            eA = ctile("eA", 2)
                            nc.scalar.activation(eA[:], dl[:], AF.Exp,
                                                 scale=Ah[j][:, n:n + 1])
                            bI = ctile("bI", 2)
                            nc.vector.tensor_tensor(bI[:], du_t[:], Bb[:], AO.mult)
                            hsc = ctile("hsc", 3)
                            init = 0.0 if c == 0 else hstate[j][:, n:n + 1]
                            nc.vector.tensor_tensor_scan(hsc[:], eA[:], bI[:],
                                                         init, AO.mult, AO.add)
                            if c < NCH - 1:
                                nc.vector.tensor_copy(hstate[j][:, n:n + 1],
                                                      hsc[:, W - 1:W])
                            nc.vector.tensor_tensor(hsc[:], hsc[:], Cb[:], AO.mult)
                            nc.vector.tensor_tensor(y_t[:], y_t[:], hsc[:], AO.add)
                        nc.vector.tensor_tensor(y_t[:], y_t[:], zs[j][:], AO.mult)
                        for mi in range(KM):
                            nc.tensor.matmul(psd[mi][:],
                                             outw[j][:, 128 * mi:128 * (mi + 1)],
                                             y_t[:], start=(j == 0),
                                             stop=(j == KD - 1))
                    for mi in range(KM):
                        pre = ctile("pre", 2)
                        nc.vector.scalar_tensor_tensor(pre[:], hch[mi][:], 0.5,
                                                       psd[mi][:], AO.mult, AO.add)
                        nc.sync.dma_start(cc_in[128 * mi:128 * (mi + 1), sl:sl + W],
                                          pre[:])

                cc_out = drp.tile([DM, T], F32, tag="cc", name="cc")
                nc.gpsimd.collective_compute(
                    "AllReduce", AO.add,
                    replica_groups=[[0, 1], [2, 3], [4, 5], [6, 7]],
                    ins=[cc_in.opt()], outs=[cc_out.opt()])
                hsrc = cc_out

            # ---------------- head ----------------
            for c in range(NCH):
                sl = c * W
                hch = []
                for j in range(KM):
                    t = ctile("hch", 5)
                    nc.sync.dma_start(t[:], hsrc[128 * j:128 * (j + 1), sl:sl + W])
                    hch.append(t)
                hf = layernorm(hch, 'hn_g', 'hn_b', 0, "hn")
                p = psA.tile([128, W], F32, tag="mm", name="mm")
                for k in range(KM):
                    nc.tensor.matmul(p[:], h1w[k][:], hf[k][:],
                                     start=(k == 0), stop=(k == KM - 1))
                c1 = ctile("c1", 1)
                nc.scalar.activation(c1[:], p[:], AF.Silu, bias=bcol('h1_b', 0))
                p2 = psC.tile([NCLS, W], F32, tag="xp", name="xp")
                nc.tensor.matmul(p2[:], h2w[:], c1[:], start=True, stop=True)
                ot = ctile("ot", 1, (NCLS, W))
                nc.vector.tensor_scalar(ot[:], p2[:], h2b[:], None, AO.add)
                nc.sync.dma_start(d_out[:, sl:sl + W], ot[:])

    _split_sync_waits(nc)
    return nc


# -------------------------------------------------------------- host side ---
def _selmat():
    s = np.zeros((48, 16 * 128), np.float32)
    for i in range(16):
        s[DR + i, 128 * i:128 * (i + 1)] = 1.0
    return s


def host_prep(inputs):
    """Build the 8 per-core input maps from the full model inputs."""
    f32 = np.float32
    g = {k: np.asarray(v, dtype=f32) for k, v in inputs.items()}
    A = -np.exp(g['A_log'])                      # [NL, DI, N]

    bias_pack = np.zeros((128, _NBC), f32)

    def put(name, vec, j=0):
        bias_pack[:, _bpcol(name, j)] = vec

    for j in range(4):
        put('fc1_b', g['fc1_b'][128 * j:128 * (j + 1)], j)
    for j in range(2):
        put('fc2_b', g['fc2_b'][128 * j:128 * (j + 1)], j)
    for j in range(4):
        put('proj_b', g['proj_b'][128 * j:128 * (j + 1)], j)
    put('h1_b', g['h1_b'], 0)
    bias_pack[:, _bpcol('eps')] = EPS
    for l in range(NL):
        for j in range(KM):
            put('ln_g', g['norm_g'][l][128 * j:128 * (j + 1)], l * KM + j)
            put('ln_b', g['norm_b'][l][128 * j:128 * (j + 1)], l * KM + j)
    for j in range(KM):
        put('hn_g', g['hn_g'][128 * j:128 * (j + 1)], j)
        put('hn_b', g['hn_b'][128 * j:128 * (j + 1)], j)

    shared = {
        'fc1_wT': np.ascontiguousarray(g['fc1_w'].T),
        'fc2_wT': np.ascontiguousarray(g['fc2_w'].T),
        'proj_wT': np.ascontiguousarray(g['proj_w'].T),
        'bias_pack': bias_pack,
        'h1_wT': np.ascontiguousarray(g['h1_w'].T),
        'h2_wT': np.ascontiguousarray(g['h2_w'].T),
        'h2_b': g['h2_b'].reshape(NCLS, 1).copy(),
        'selmat': _selmat(),
    }
    for l in range(NL):
        shared[f'inwT{l}'] = np.ascontiguousarray(g['in_w'][l].T)
        shared[f'outwT{l}'] = np.ascontiguousarray(g['out_w'][l].T)
        shared[f'dtwT{l}'] = np.ascontiguousarray(g['dt_w'][l].T)
        cp = np.zeros((DI, 7), f32)
        cp[:, 0:4] = g['conv_w'][l]
        cp[:, 4] = g['conv_b'][l]
        cp[:, 5] = g['dt_b'][l]
        cp[:, 6] = 0.5 * g['D_skip'][l]          # both halves add half of D*uc
        shared[f'cpak{l}'] = cp

    in_maps = []
    for c in range(8):
        b, q = c // 2, c % 2
        m = dict(shared)
        m['xb'] = np.ascontiguousarray(g['x'][b])
        for l in range(NL):
            rows = list(range(DR)) \
                 + [DR + q * 8 + i for i in range(8)] \
                 + [DR + N + q * 8 + i for i in range(8)]
            m[f'xpjT{l}'] = np.ascontiguousarray(g['xproj_w'][l][rows].T)
            m[f'Ah{l}'] = np.ascontiguousarray(A[l][:, q * 8:q * 8 + 8])
        in_maps.append(m)
    return in_maps


_NC_CACHE = {}


def _get_nc():
    if 'nc' not in _NC_CACHE:
        _NC_CACHE['nc'] = build_nc()
    return _NC_CACHE['nc']


def kernel(**inputs):
    nc = _get_nc()
    in_maps = host_prep(inputs)
    res = run_bass_kernel_spmd(nc, in_maps, core_ids=list(range(8)))
    out = np.stack([res.results[2 * b]['out'] for b in range(B)])
    return out.astype(np.float32)


def run_traced(inputs, **kw):
    """test.py helper: run with NTFF tracing, return (output, results)."""
    nc = _get_nc()
    in_maps = host_prep(inputs)
    res = run_bass_kernel_spmd(nc, in_maps, core_ids=list(range(8)),
                               trace=True, **kw)
    out = np.stack([res.results[2 * b]['out'] for b in range(B)])
    return out.astype(np.float32), res
